# revision 6
# baseline (speedup 1.0000x reference)
"""Multi-head self-attention (B=2, N=4096, C=512, H=8) on 8 trn2 NeuronCores.

Sharding: one head per core (tensor parallel over heads). Each core computes
Q^T/K^T/V for its head from the full token stream, runs flash-style attention
(S^T = K Q^T row-tiled pairs, exp with the 1/sqrt(Dh) scale folded in,
P^T @ V accumulated in PSUM with an appended ones-column producing softmax
denominators), and projects through the head's w_proj slice, writing an
UNNORMALIZED [8192, 512] fp32 partial plus per-token denominators. The host
divides each partial by its denominators, sums the 8 results and adds b_proj.

Performance structure (the wall is ScalarE: 33.5M exps/core at 1 elem/cycle/
lane ~= 218us + per-instruction overhead):
  - QKV prep is pipelined with the exp streams of the first TWO query blocks
    so ScalarE saturates within a few microseconds.
  - A fraction of S^T tiles is relayed PSUM->SBUF by VectorE and exp'd in
    wide [128, 4096] ACTs (overhead amortized 4x), splitting the exp cost
    between the per-instruction overhead (ScalarE) and the copy (VectorE).
  - A*V matmuls run a generation (query block) behind the QK/exp stream,
    draining at a controlled rate so the PE never head-of-line-blocks the
    exp stream; the lag decays near the end to shrink the tail.
  - Every PSUM tile is bank-exclusive (sub-bank sharing serializes PE writes
    against DVE reads via Tile's bank-aware tracker).
"""

import numpy as np
import ml_dtypes

import concourse.bacc as bacc
import concourse.bass as bass
import concourse.mybir as mybir
import concourse.tile as tile
from concourse.bass_utils import run_bass_kernel_spmd

BF16 = ml_dtypes.bfloat16

B = 2
N = 4096          # sequence length per batch
C = 512           # channels
H = 8             # heads
DH = C // H       # 64 head dim
T = B * N         # total tokens
NB = 512          # query-block size
MC = 128          # key-chunk size
SCALE = float(DH) ** -0.5

PERIOD = 8        # relay pattern period in mc-steps
RELAY = 4         # of each PERIOD, this many steps exp via SBUF relay


def _emit(nc, tc, xt, wq, wk, wv, wp, out, out_den, n_seq):
    dt = mybir.dt
    cch = C // 128             # contraction chunks over C (4)
    nblk = n_seq // NB         # query blocks per batch (8)
    nmc = n_seq // MC          # key chunks per batch (32)
    ntc = n_seq // 512         # 512-token prep chunks per batch (8)
    mpc = 512 // MC            # key chunks per prep chunk (4)
    tpb = NB // 128            # 128-token proj chunks per query block (4)
    EXP = mybir.ActivationFunctionType.Exp

    const = tc.alloc_tile_pool(name="const", bufs=1)
    persist = tc.alloc_tile_pool(name="persist", bufs=1)

    # ---------------- constants ----------------
    wq_sb = const.tile([128, cch, DH], dt.bfloat16)
    wk_sb = const.tile([128, cch, DH], dt.bfloat16)
    wv_sb = const.tile([128, cch, DH], dt.bfloat16)
    nc.sync.dma_start(wq_sb[:], wq.rearrange("(c p) d -> p c d", p=128))
    nc.sync.dma_start(wk_sb[:], wk.rearrange("(c p) d -> p c d", p=128))
    nc.sync.dma_start(wv_sb[:], wv.rearrange("(c p) d -> p c d", p=128))
    wp_sb = const.tile([DH, C], dt.bfloat16)
    nc.sync.dma_start(wp_sb[:], wp)
    zbias = const.tile([128, 1], dt.float32)
    nc.vector.memset(zbias[:], 0.0)

    # ---------------- persistent per-head tensors ----------------
    qt2 = persist.tile([128, n_seq], dt.bfloat16)   # rows 0:64 batch0, 64:128 batch1
    kt2 = persist.tile([128, n_seq], dt.bfloat16)
    vext = [persist.tile([128, nmc * (DH + 1)], dt.bfloat16, name=f"vext{j}")
            for j in range(2)]
    otsb = [persist.tile([DH, n_seq], dt.bfloat16, name=f"otsb{j}")
            for j in range(2)]
    for j in range(2):
        ones_ap = vext[j][:].rearrange("p (t c) -> p t c", c=DH + 1)[:, :, DH:DH + 1]
        nc.vector.memset(ones_ap, 1.0)

    # ---------------- PSUM pools (8 banks) ----------------
    spool = tc.alloc_tile_pool(name="sps", bufs=2, space="PSUM")   # 2x2 banks
    apool = tc.alloc_tile_pool(name="aps", bufs=1, space="PSUM")   # 2 banks
    pps = tc.alloc_tile_pool(name="pps", bufs=1, space="PSUM")     # 2 banks

    # ---------------- SBUF pools ----------------
    xpool = tc.alloc_tile_pool(name="xa", bufs=6)
    ptp = tc.alloc_tile_pool(name="ptp", bufs=20)
    sgip = tc.alloc_tile_pool(name="sgi", bufs=2)
    sgop = tc.alloc_tile_pool(name="sgo", bufs=6)
    eps = tc.alloc_tile_pool(name="eps", bufs=2)

    # warm the exp table set (~2.7us ACT_TABLE_LOAD) during prep c0
    warm = const.tile([128, 1], dt.float32)
    nc.scalar.activation(warm[:], zbias[:], EXP, bias=zbias[:], scale=1.0)

    # ---------------- prep ----------------
    xtiles = {}

    def emit_xload(c):
        xa = xpool.tile([128, cch, 512], dt.bfloat16, tag="x", name="xa")
        xb = xpool.tile([128, cch, 512], dt.bfloat16, tag="x", name="xb")
        nc.sync.dma_start(
            xa[:], xt[:, c * 512:(c + 1) * 512].rearrange("(k p) i -> p k i", p=128))
        nc.sync.dma_start(
            xb[:], xt[:, n_seq + c * 512:n_seq + (c + 1) * 512].rearrange(
                "(k p) i -> p k i", p=128))
        xtiles[c] = (xa, xb)

    def prep_chunk(c):
        """Q^T/K^T (col-packed batch pairs) + V for 512-token chunk c."""
        xa, xb = xtiles.pop(c)
        psq = pps.tile([128, 512], dt.float32, tag="qk", name="psq")
        for k in range(cch):
            nc.tensor.matmul(psq[0:DH, :], wq_sb[:, k, :], xa[:, k, :],
                             start=(k == 0), stop=(k == cch - 1),
                             tile_position=(0, 0))
            nc.tensor.matmul(psq[DH:2 * DH, :], wq_sb[:, k, :], xb[:, k, :],
                             start=(k == 0), stop=(k == cch - 1),
                             tile_position=(0, 64))
        psv0 = pps.tile([128, 256], dt.float32, tag="v", name="psv0",
                        padded_shape=(128, 512))
        for mt in range(4):
            for k in range(cch):
                nc.tensor.matmul(psv0[:, mt * DH:(mt + 1) * DH],
                                 xa[:, k, mt * 128:(mt + 1) * 128],
                                 wv_sb[:, k, :],
                                 start=(k == 0), stop=(k == cch - 1))
        nc.vector.tensor_copy(qt2[:, c * 512:(c + 1) * 512], psq[:])
        vdst0 = vext[0][:].rearrange("p (t c2) -> p t c2", c2=DH + 1)[
            :, mpc * c:mpc * (c + 1), 0:DH]
        nc.vector.tensor_copy(vdst0, psv0[:].rearrange("p (t d) -> p t d", d=DH))
        psk = pps.tile([128, 512], dt.float32, tag="qk", name="psk")
        for k in range(cch):
            nc.tensor.matmul(psk[0:DH, :], wk_sb[:, k, :], xa[:, k, :],
                             start=(k == 0), stop=(k == cch - 1),
                             tile_position=(0, 0))
            nc.tensor.matmul(psk[DH:2 * DH, :], wk_sb[:, k, :], xb[:, k, :],
                             start=(k == 0), stop=(k == cch - 1),
                             tile_position=(0, 64))
        psv1 = pps.tile([128, 256], dt.float32, tag="v", name="psv1",
                        padded_shape=(128, 512))
        for mt in range(4):
            for k in range(cch):
                nc.tensor.matmul(psv1[:, mt * DH:(mt + 1) * DH],
                                 xb[:, k, mt * 128:(mt + 1) * 128],
                                 wv_sb[:, k, :],
                                 start=(k == 0), stop=(k == cch - 1))
        nc.vector.tensor_copy(kt2[:, c * 512:(c + 1) * 512], psk[:])
        vdst1 = vext[1][:].rearrange("p (t c2) -> p t c2", c2=DH + 1)[
            :, mpc * c:mpc * (c + 1), 0:DH]
        nc.vector.tensor_copy(vdst1, psv1[:].rearrange("p (t d) -> p t d", d=DH))

    # ---------------- attention stream machinery ----------------
    # pend[gen] holds (mc, ap) for emitted-but-not-AV'd steps; AVs drain the
    # oldest generation only (acc tiles ring with bufs=1: generations must not
    # interleave). relay runs live in rstate while being filled.
    pend = {g: [] for g in range(nblk)}
    pushed = [0] * nblk
    state = {"pop_gen": 0, "acc": None, "proj_q": [], "jps": None}
    rstate = {}

    def emit_qk_only(nb, mc):
        st = spool.tile([128, 1024], dt.float32, tag="s", name="st")
        for j in range(2):
            nc.tensor.matmul(
                st[:, j * 512:j * 512 + NB],
                kt2[j * DH:(j + 1) * DH, mc * MC:(mc + 1) * MC],
                qt2[j * DH:(j + 1) * DH, nb * NB:(nb + 1) * NB],
                start=True, stop=True,
                tile_position=(j * DH, 0))
        return st

    def emit_step(nb, mc):
        """QK matmul pair + exp (direct or relayed) for (nb, mc)."""
        st = emit_qk_only(nb, mc)
        r = mc % PERIOD - (PERIOD - RELAY)
        if r < 0:     # direct: exp straight from PSUM
            pt = ptp.tile([128, 1024], dt.bfloat16, tag="pt", name="pt")
            nc.scalar.activation(pt[:], st[:], EXP, bias=zbias[:], scale=SCALE)
            pend[nb].append((mc, pt[:]))
        else:         # relayed: DVE copy to SBUF, wide exp at run end
            if r == 0:
                sgi = sgip.tile([128, RELAY, 1024], dt.float32, tag="sgi",
                                name="sgi")
                sgo = sgop.tile([128, RELAY, 1024], dt.bfloat16, tag="sgo",
                                name="sgo")
                rstate[nb] = (sgi, sgo)
            sgi, sgo = rstate[nb]
            nc.vector.tensor_copy(sgi[:, r, :], st[:])
            pend[nb].append((mc, sgo[:, r, :]))
            if r == RELAY - 1:
                nc.scalar.activation(
                    sgo[:].rearrange("p a b -> p (a b)"),
                    sgi[:].rearrange("p a b -> p (a b)"),
                    EXP, bias=zbias[:], scale=SCALE)
        pushed[nb] += 1

    def pop_av():
        """Emit one A*V pair from the oldest generation. Returns False if
        nothing is poppable."""
        g = state["pop_gen"]
        if g >= nblk or not pend[g]:
            return False
        mc, ap = pend[g].pop(0)
        if mc == 0:
            state["acc"] = [apool.tile([DH + 1, NB], dt.float32, tag=f"acc{j}",
                                       name=f"acc{j}") for j in range(2)]
        acc = state["acc"]
        first, last = (mc == 0), (mc == nmc - 1)
        for j in range(2):
            nc.tensor.matmul(
                acc[j][:],
                vext[j][:, mc * (DH + 1):(mc + 1) * (DH + 1)],
                ap[:, j * 512:j * 512 + NB],
                start=first, stop=last)
        if last:
            emit_epilogue(g, acc)
            state["proj_q"].extend((g, u) for u in range(2 * tpb))
            state["pop_gen"] = g + 1
        return True

    def emit_epilogue(nb, acc):
        """Spill unnormalized head output; DMA denominators straight out."""
        lst = eps.tile([DH + 1, 2 * NB], dt.float32, tag="ls", name="lst")
        for j in range(2):
            nc.vector.tensor_copy(otsb[j][:, nb * NB:(nb + 1) * NB],
                                  acc[j][0:DH, :])
            nc.vector.tensor_copy(lst[DH:DH + 1, j * NB:(j + 1) * NB],
                                  acc[j][DH:DH + 1, :])
        nc.sync.dma_start(
            out_den[:, nb * NB:(nb + 1) * NB],
            lst[DH:DH + 1, :].rearrange("o (j n) -> o j n", j=2))

    def emit_proj_unit():
        if not state["proj_q"]:
            return
        nb, u = state["proj_q"].pop(0)
        j, t = divmod(u, tpb)
        gt = nb * tpb + t
        pp = state["jps"].tile([128, C], dt.float32, tag="pp", name="pp")
        nc.tensor.matmul(pp[:], otsb[j][:, gt * 128:(gt + 1) * 128],
                         wp_sb[:], start=True, stop=True)
        ob = eps.tile([128, C], dt.float32, tag="ob", name="ob")
        nc.vector.tensor_copy(ob[:], pp[:])
        nc.sync.dma_start(
            out[j * n_seq + gt * 128: j * n_seq + (gt + 1) * 128, :], ob[:])

    def backlog():
        g = state["pop_gen"]
        return sum(len(pend[g2]) for g2 in range(g, nblk))

    # ---------------- phase A: prep + exp streams of nb=0 and nb=1 ----------
    emit_xload(0)
    emit_xload(1)
    for c in range(ntc):
        if c + 2 < ntc:
            emit_xload(c + 2)
        prep_chunk(c)
        for i in range(mpc):
            emit_step(0, mpc * c + i)
            if len(pend[0]) > 8 and state["pop_gen"] == 0:
                pop_av()
            if c >= 1:
                emit_step(1, mpc * (c - 1) + i)
                if len(pend[0]) > 8 and state["pop_gen"] == 0:
                    pop_av()
    for i in range(mpc):
        emit_step(1, mpc * (ntc - 1) + i)
        if state["pop_gen"] == 0:
            pop_av()
    while state["pop_gen"] == 0:
        pop_av()
    pps.release()

    # ---------------- phase B: streams nb=2..7, gen-lagged AV drain ---------
    state["jps"] = tc.alloc_tile_pool(name="jps", bufs=2, space="PSUM")
    for nb in range(2, nblk):
        for mc in range(nmc):
            emit_step(nb, mc)
            pop_av()
            if nb >= nblk - 3 and mc % 4 == 0:
                pop_av()      # decay the lag so the tail stays short
            if mc % 3 == 2:
                emit_proj_unit()
    # tail: drain remaining generations and projections
    while pop_av():
        emit_proj_unit()
    while state["proj_q"]:
        emit_proj_unit()

    state["jps"].release()
    eps.release()
    sgop.release()
    sgip.release()
    ptp.release()
    xpool.release()
    apool.release()
    spool.release()
    persist.release()
    const.release()


def build_kernel(n_seq=N):
    nc = bacc.Bacc("TRN2", target_bir_lowering=False, debug=False, num_devices=8)
    dt = mybir.dt
    t_tot = 2 * n_seq
    xt = nc.dram_tensor("xt", [C, t_tot], dt.bfloat16, kind="ExternalInput").ap()
    wq = nc.dram_tensor("wq", [C, DH], dt.bfloat16, kind="ExternalInput").ap()
    wk = nc.dram_tensor("wk", [C, DH], dt.bfloat16, kind="ExternalInput").ap()
    wv = nc.dram_tensor("wv", [C, DH], dt.bfloat16, kind="ExternalInput").ap()
    wp = nc.dram_tensor("wp", [DH, C], dt.bfloat16, kind="ExternalInput").ap()
    out = nc.dram_tensor("out", [t_tot, C], dt.float32, kind="ExternalOutput").ap()
    out_den = nc.dram_tensor("out_den", [2, n_seq], dt.float32,
                             kind="ExternalOutput").ap()
    with tile.TileContext(nc) as tc:
        _emit(nc, tc, xt, wq, wk, wv, wp, out, out_den, n_seq)
    nc.compile()
    return nc


def make_in_maps(x, w_qkv, w_proj, n_seq=N):
    """Slice the full inputs into 8 per-core input maps (head per core)."""
    t_tot = 2 * n_seq
    xt = np.ascontiguousarray(x.reshape(t_tot, C).T).astype(BF16)
    in_maps = []
    for h in range(H):
        wq = np.ascontiguousarray(w_qkv[h * DH:(h + 1) * DH, :].T).astype(BF16)
        wk = np.ascontiguousarray(w_qkv[C + h * DH:C + (h + 1) * DH, :].T).astype(BF16)
        wv = np.ascontiguousarray(
            w_qkv[2 * C + h * DH:2 * C + (h + 1) * DH, :].T).astype(BF16)
        wp = np.ascontiguousarray(w_proj[:, h * DH:(h + 1) * DH].T).astype(BF16)
        in_maps.append({"xt": xt, "wq": wq, "wk": wk, "wv": wv, "wp": wp})
    return in_maps


_NC_CACHE = {}


def _get_nc(n_seq=N):
    if n_seq not in _NC_CACHE:
        _NC_CACHE[n_seq] = build_kernel(n_seq)
    return _NC_CACHE[n_seq]


def run(x, w_qkv, w_proj, b_proj, trace=False, tmpdir=None):
    x = np.asarray(x, dtype=np.float32)
    w_qkv = np.asarray(w_qkv, dtype=np.float32)
    w_proj = np.asarray(w_proj, dtype=np.float32)
    b_proj = np.asarray(b_proj, dtype=np.float32)
    nc = _get_nc()
    in_maps = make_in_maps(x, w_qkv, w_proj)
    try:
        res = run_bass_kernel_spmd(nc, in_maps, list(range(H)), trace=trace,
                                   tmpdir=tmpdir)
    except ModuleNotFoundError:
        # no NTFF profiling hook in this environment
        res = run_bass_kernel_spmd(nc, in_maps, list(range(H)), trace=False,
                                   tmpdir=tmpdir)
    partial_sum = np.zeros((T, C), np.float64)
    for r in res.results:
        den = r["out_den"].astype(np.float64).reshape(T, 1)
        partial_sum += r["out"].astype(np.float64) / den
    full = (partial_sum + b_proj[None, :].astype(np.float64)).astype(np.float32)
    return full.reshape(B, N, C), res


def kernel(x, w_qkv, w_proj, b_proj):
    out, _ = run(x, w_qkv, w_proj, b_proj)
    return out


# revision 8
# speedup vs baseline: 1.6722x; 1.6722x over previous
"""Multi-head self-attention (B=2, N=4096, C=512, H=8) on 8 trn2 NeuronCores.

Sharding: one head per core (tensor parallel over heads). Each core computes
Q^T/K^T/V for its head from the full token stream, runs flash-style attention
(S^T = K Q^T row-tiled pairs, exp with the 1/sqrt(Dh) scale folded in,
P^T @ V accumulated in PSUM with an appended ones-column producing softmax
denominators), and projects through the head's w_proj slice, writing an
UNNORMALIZED [8192, 512] fp32 partial plus per-token denominators. The host
divides each partial by its denominators, sums the 8 results and adds b_proj.

Performance structure (the wall is ScalarE: 33.5M exps/core at 1 elem/cycle/
lane ~= 218us + per-instruction overhead):
  - QKV prep is pipelined with the exp streams of the first TWO query blocks
    so ScalarE saturates within a few microseconds.
  - A fraction of S^T tiles is relayed PSUM->SBUF by VectorE and exp'd in
    wide [128, 4096] ACTs (overhead amortized 4x), splitting the exp cost
    between the per-instruction overhead (ScalarE) and the copy (VectorE).
  - A*V matmuls run a generation (query block) behind the QK/exp stream,
    draining at a controlled rate so the PE never head-of-line-blocks the
    exp stream; the lag decays near the end to shrink the tail.
  - Every PSUM tile is bank-exclusive (sub-bank sharing serializes PE writes
    against DVE reads via Tile's bank-aware tracker).
"""

import numpy as np
import ml_dtypes

import concourse.bacc as bacc
import concourse.bass as bass
import concourse.mybir as mybir
import concourse.tile as tile
from concourse.bass_utils import run_bass_kernel_spmd

BF16 = ml_dtypes.bfloat16

B = 2
N = 4096          # sequence length per batch
C = 512           # channels
H = 8             # heads
DH = C // H       # 64 head dim
T = B * N         # total tokens
NB = 512          # query-block size
MC = 128          # key-chunk size
SCALE = float(DH) ** -0.5

PERIOD = 8        # relay pattern period in mc-steps
RELAY = 0         # of each PERIOD, this many steps exp via SBUF relay
                  # (measured: SBUF-source ACTs run ~1.3 cyc/elem, erasing the
                  # overhead amortization — relay disabled)


def _emit(nc, tc, xt, wq, wk, wv, wp, out, out_den, n_seq):
    dt = mybir.dt
    cch = C // 128             # contraction chunks over C (4)
    nblk = n_seq // NB         # query blocks per batch (8)
    nmc = n_seq // MC          # key chunks per batch (32)
    ntc = n_seq // 512         # 512-token prep chunks per batch (8)
    mpc = 512 // MC            # key chunks per prep chunk (4)
    tpb = NB // 128            # 128-token proj chunks per query block (4)
    EXP = mybir.ActivationFunctionType.Exp

    const = tc.alloc_tile_pool(name="const", bufs=1)
    persist = tc.alloc_tile_pool(name="persist", bufs=1)

    # ---------------- constants ----------------
    wq_sb = const.tile([128, cch, DH], dt.bfloat16)
    wk_sb = const.tile([128, cch, DH], dt.bfloat16)
    wv_sb = const.tile([128, cch, DH], dt.bfloat16)
    nc.sync.dma_start(wq_sb[:], wq.rearrange("(c p) d -> p c d", p=128))
    nc.sync.dma_start(wk_sb[:], wk.rearrange("(c p) d -> p c d", p=128))
    nc.sync.dma_start(wv_sb[:], wv.rearrange("(c p) d -> p c d", p=128))
    wp_sb = const.tile([DH, C], dt.bfloat16)
    nc.sync.dma_start(wp_sb[:], wp)
    zbias = const.tile([128, 1], dt.float32)
    nc.vector.memset(zbias[:], 0.0)

    # ---------------- persistent per-head tensors ----------------
    qt2 = persist.tile([128, n_seq], dt.bfloat16)   # rows 0:64 batch0, 64:128 batch1
    kt2 = persist.tile([128, n_seq], dt.bfloat16)
    vext = [persist.tile([128, nmc * (DH + 1)], dt.bfloat16, name=f"vext{j}")
            for j in range(2)]
    otsb = [persist.tile([DH, n_seq], dt.bfloat16, name=f"otsb{j}")
            for j in range(2)]
    for j in range(2):
        ones_ap = vext[j][:].rearrange("p (t c) -> p t c", c=DH + 1)[:, :, DH:DH + 1]
        nc.vector.memset(ones_ap, 1.0)

    # ---------------- PSUM pools (8 banks) ----------------
    spool = tc.alloc_tile_pool(name="sps", bufs=2, space="PSUM")   # 2x2 banks
    apool = tc.alloc_tile_pool(name="aps", bufs=1, space="PSUM")   # 2 banks
    pps = tc.alloc_tile_pool(name="pps", bufs=1, space="PSUM")     # 2 banks

    # ---------------- SBUF pools ----------------
    xpool = tc.alloc_tile_pool(name="xa", bufs=6)
    # pt ring must exceed the AV generation-lag (32 steps) plus S-ring slack,
    # else ACT(pt-slot reuse) -> AV -> PE-queue -> QK -> ACT deadlocks.
    ptp = tc.alloc_tile_pool(name="ptp", bufs=38)
    sgip = tc.alloc_tile_pool(name="sgi", bufs=2)
    sgop = tc.alloc_tile_pool(name="sgo", bufs=6)
    eps = tc.alloc_tile_pool(name="eps", bufs=2)

    # warm the exp table set (~2.7us ACT_TABLE_LOAD) during prep c0
    warm = const.tile([128, 1], dt.float32)
    nc.scalar.activation(warm[:], zbias[:], EXP, bias=zbias[:], scale=1.0)

    # ---------------- prep ----------------
    xtiles = {}

    def emit_xload(c):
        xa = xpool.tile([128, cch, 512], dt.bfloat16, tag="x", name="xa")
        xb = xpool.tile([128, cch, 512], dt.bfloat16, tag="x", name="xb")
        nc.sync.dma_start(
            xa[:], xt[:, c * 512:(c + 1) * 512].rearrange("(k p) i -> p k i", p=128))
        nc.sync.dma_start(
            xb[:], xt[:, n_seq + c * 512:n_seq + (c + 1) * 512].rearrange(
                "(k p) i -> p k i", p=128))
        xtiles[c] = (xa, xb)

    def prep_chunk(c):
        """Q^T/K^T (col-packed batch pairs) + V for 512-token chunk c."""
        xa, xb = xtiles.pop(c)
        psq = pps.tile([128, 512], dt.float32, tag="qk", name="psq")
        for k in range(cch):
            nc.tensor.matmul(psq[0:DH, :], wq_sb[:, k, :], xa[:, k, :],
                             start=(k == 0), stop=(k == cch - 1),
                             tile_position=(0, 0))
            nc.tensor.matmul(psq[DH:2 * DH, :], wq_sb[:, k, :], xb[:, k, :],
                             start=(k == 0), stop=(k == cch - 1),
                             tile_position=(0, 64))
        psv0 = pps.tile([128, 256], dt.float32, tag="v", name="psv0",
                        padded_shape=(128, 512))
        for mt in range(4):
            for k in range(cch):
                nc.tensor.matmul(psv0[:, mt * DH:(mt + 1) * DH],
                                 xa[:, k, mt * 128:(mt + 1) * 128],
                                 wv_sb[:, k, :],
                                 start=(k == 0), stop=(k == cch - 1))
        nc.vector.tensor_copy(qt2[:, c * 512:(c + 1) * 512], psq[:])
        vdst0 = vext[0][:].rearrange("p (t c2) -> p t c2", c2=DH + 1)[
            :, mpc * c:mpc * (c + 1), 0:DH]
        nc.vector.tensor_copy(vdst0, psv0[:].rearrange("p (t d) -> p t d", d=DH))
        psk = pps.tile([128, 512], dt.float32, tag="qk", name="psk")
        for k in range(cch):
            nc.tensor.matmul(psk[0:DH, :], wk_sb[:, k, :], xa[:, k, :],
                             start=(k == 0), stop=(k == cch - 1),
                             tile_position=(0, 0))
            nc.tensor.matmul(psk[DH:2 * DH, :], wk_sb[:, k, :], xb[:, k, :],
                             start=(k == 0), stop=(k == cch - 1),
                             tile_position=(0, 64))
        psv1 = pps.tile([128, 256], dt.float32, tag="v", name="psv1",
                        padded_shape=(128, 512))
        for mt in range(4):
            for k in range(cch):
                nc.tensor.matmul(psv1[:, mt * DH:(mt + 1) * DH],
                                 xb[:, k, mt * 128:(mt + 1) * 128],
                                 wv_sb[:, k, :],
                                 start=(k == 0), stop=(k == cch - 1))
        nc.vector.tensor_copy(kt2[:, c * 512:(c + 1) * 512], psk[:])
        vdst1 = vext[1][:].rearrange("p (t c2) -> p t c2", c2=DH + 1)[
            :, mpc * c:mpc * (c + 1), 0:DH]
        nc.vector.tensor_copy(vdst1, psv1[:].rearrange("p (t d) -> p t d", d=DH))

    # ---------------- attention stream machinery ----------------
    # pend[gen] holds (mc, ap) for emitted-but-not-AV'd steps; AVs drain the
    # oldest generation only (acc tiles ring with bufs=1: generations must not
    # interleave). relay runs live in rstate while being filled.
    pend = {g: [] for g in range(nblk)}
    pushed = [0] * nblk
    state = {"pop_gen": 0, "acc": None, "proj_q": [], "jps": None}
    rstate = {}

    def emit_qk_only(nb, mc):
        st = spool.tile([128, 1024], dt.float32, tag="s", name="st")
        for j in range(2):
            nc.tensor.matmul(
                st[:, j * 512:j * 512 + NB],
                kt2[j * DH:(j + 1) * DH, mc * MC:(mc + 1) * MC],
                qt2[j * DH:(j + 1) * DH, nb * NB:(nb + 1) * NB],
                start=True, stop=True,
                tile_position=(j * DH, 0))
        return st

    def emit_step(nb, mc):
        """QK matmul pair + exp (direct or relayed) for (nb, mc)."""
        st = emit_qk_only(nb, mc)
        r = mc % PERIOD - (PERIOD - RELAY)
        if r < 0:     # direct: exp straight from PSUM
            pt = ptp.tile([128, 1024], dt.bfloat16, tag="pt", name="pt")
            nc.scalar.activation(pt[:], st[:], EXP, bias=zbias[:], scale=SCALE)
            pend[nb].append((mc, pt[:]))
        else:         # relayed: DVE copy to SBUF, wide exp at run end
            if r == 0:
                sgi = sgip.tile([128, RELAY, 1024], dt.float32, tag="sgi",
                                name="sgi")
                sgo = sgop.tile([128, RELAY, 1024], dt.bfloat16, tag="sgo",
                                name="sgo")
                rstate[nb] = (sgi, sgo)
            sgi, sgo = rstate[nb]
            nc.vector.tensor_copy(sgi[:, r, :], st[:])
            pend[nb].append((mc, sgo[:, r, :]))
            if r == RELAY - 1:
                nc.scalar.activation(
                    sgo[:].rearrange("p a b -> p (a b)"),
                    sgi[:].rearrange("p a b -> p (a b)"),
                    EXP, bias=zbias[:], scale=SCALE)
        pushed[nb] += 1

    def pop_av():
        """Emit one A*V pair from the oldest generation. Returns False if
        nothing is poppable."""
        g = state["pop_gen"]
        if g >= nblk or not pend[g]:
            return False
        mc, ap = pend[g].pop(0)
        if mc == 0:
            state["acc"] = [apool.tile([DH + 1, NB], dt.float32, tag=f"acc{j}",
                                       name=f"acc{j}") for j in range(2)]
        acc = state["acc"]
        first, last = (mc == 0), (mc == nmc - 1)
        for j in range(2):
            nc.tensor.matmul(
                acc[j][:],
                vext[j][:, mc * (DH + 1):(mc + 1) * (DH + 1)],
                ap[:, j * 512:j * 512 + NB],
                start=first, stop=last)
        if last:
            emit_epilogue(g, acc)
            state["proj_q"].extend((g, u) for u in range(2 * tpb))
            state["pop_gen"] = g + 1
        return True

    def emit_epilogue(nb, acc):
        """Spill unnormalized head output; DMA denominators straight out."""
        lst = eps.tile([DH + 1, 2 * NB], dt.float32, tag="ls", name="lst")
        for j in range(2):
            nc.vector.tensor_copy(otsb[j][:, nb * NB:(nb + 1) * NB],
                                  acc[j][0:DH, :])
            nc.vector.tensor_copy(lst[DH:DH + 1, j * NB:(j + 1) * NB],
                                  acc[j][DH:DH + 1, :])
        nc.sync.dma_start(
            out_den[:, nb * NB:(nb + 1) * NB],
            lst[DH:DH + 1, :].rearrange("o (j n) -> o j n", j=2))

    def emit_proj_unit():
        if not state["proj_q"]:
            return
        nb, u = state["proj_q"].pop(0)
        j, t = divmod(u, tpb)
        gt = nb * tpb + t
        pp = state["jps"].tile([128, C], dt.float32, tag="pp", name="pp")
        nc.tensor.matmul(pp[:], otsb[j][:, gt * 128:(gt + 1) * 128],
                         wp_sb[:], start=True, stop=True)
        ob = eps.tile([128, C], dt.float32, tag="ob", name="ob")
        nc.vector.tensor_copy(ob[:], pp[:])
        nc.sync.dma_start(
            out[j * n_seq + gt * 128: j * n_seq + (gt + 1) * 128, :], ob[:])

    def backlog():
        g = state["pop_gen"]
        return sum(len(pend[g2]) for g2 in range(g, nblk))

    # ---------------- phase A: prep + exp streams of nb=0 and nb=1 ----------
    emit_xload(0)
    emit_xload(1)
    for c in range(ntc):
        if c + 2 < ntc:
            emit_xload(c + 2)
        prep_chunk(c)
        for i in range(mpc):
            emit_step(0, mpc * c + i)
            if len(pend[0]) > 8 and state["pop_gen"] == 0:
                pop_av()
            if c >= 1:
                emit_step(1, mpc * (c - 1) + i)
                if len(pend[0]) > 8 and state["pop_gen"] == 0:
                    pop_av()
    for i in range(mpc):
        emit_step(1, mpc * (ntc - 1) + i)
        if state["pop_gen"] == 0:
            pop_av()
    while state["pop_gen"] == 0:
        pop_av()
    pps.release()

    # ---------------- phase B: streams nb=2..7, gen-lagged AV drain ---------
    state["jps"] = tc.alloc_tile_pool(name="jps", bufs=2, space="PSUM")
    for nb in range(2, nblk):
        for mc in range(nmc):
            emit_step(nb, mc)
            pop_av()
            if nb >= nblk - 3 and mc % 4 == 0:
                pop_av()      # decay the lag so the tail stays short
            if mc % 3 == 2:
                emit_proj_unit()
    # tail: drain remaining generations and projections
    while pop_av():
        emit_proj_unit()
    while state["proj_q"]:
        emit_proj_unit()

    state["jps"].release()
    eps.release()
    sgop.release()
    sgip.release()
    ptp.release()
    xpool.release()
    apool.release()
    spool.release()
    persist.release()
    const.release()


def build_kernel(n_seq=N):
    nc = bacc.Bacc("TRN2", target_bir_lowering=False, debug=False, num_devices=8)
    dt = mybir.dt
    t_tot = 2 * n_seq
    xt = nc.dram_tensor("xt", [C, t_tot], dt.bfloat16, kind="ExternalInput").ap()
    wq = nc.dram_tensor("wq", [C, DH], dt.bfloat16, kind="ExternalInput").ap()
    wk = nc.dram_tensor("wk", [C, DH], dt.bfloat16, kind="ExternalInput").ap()
    wv = nc.dram_tensor("wv", [C, DH], dt.bfloat16, kind="ExternalInput").ap()
    wp = nc.dram_tensor("wp", [DH, C], dt.bfloat16, kind="ExternalInput").ap()
    out = nc.dram_tensor("out", [t_tot, C], dt.float32, kind="ExternalOutput").ap()
    out_den = nc.dram_tensor("out_den", [2, n_seq], dt.float32,
                             kind="ExternalOutput").ap()
    with tile.TileContext(nc) as tc:
        _emit(nc, tc, xt, wq, wk, wv, wp, out, out_den, n_seq)
    nc.compile()
    return nc


def make_in_maps(x, w_qkv, w_proj, n_seq=N):
    """Slice the full inputs into 8 per-core input maps (head per core)."""
    t_tot = 2 * n_seq
    xt = np.ascontiguousarray(x.reshape(t_tot, C).T).astype(BF16)
    in_maps = []
    for h in range(H):
        wq = np.ascontiguousarray(w_qkv[h * DH:(h + 1) * DH, :].T).astype(BF16)
        wk = np.ascontiguousarray(w_qkv[C + h * DH:C + (h + 1) * DH, :].T).astype(BF16)
        wv = np.ascontiguousarray(
            w_qkv[2 * C + h * DH:2 * C + (h + 1) * DH, :].T).astype(BF16)
        wp = np.ascontiguousarray(w_proj[:, h * DH:(h + 1) * DH].T).astype(BF16)
        in_maps.append({"xt": xt, "wq": wq, "wk": wk, "wv": wv, "wp": wp})
    return in_maps


_NC_CACHE = {}


def _get_nc(n_seq=N):
    if n_seq not in _NC_CACHE:
        _NC_CACHE[n_seq] = build_kernel(n_seq)
    return _NC_CACHE[n_seq]


def run(x, w_qkv, w_proj, b_proj, trace=False, tmpdir=None):
    x = np.asarray(x, dtype=np.float32)
    w_qkv = np.asarray(w_qkv, dtype=np.float32)
    w_proj = np.asarray(w_proj, dtype=np.float32)
    b_proj = np.asarray(b_proj, dtype=np.float32)
    nc = _get_nc()
    in_maps = make_in_maps(x, w_qkv, w_proj)
    try:
        res = run_bass_kernel_spmd(nc, in_maps, list(range(H)), trace=trace,
                                   tmpdir=tmpdir)
    except ModuleNotFoundError:
        # no NTFF profiling hook in this environment
        res = run_bass_kernel_spmd(nc, in_maps, list(range(H)), trace=False,
                                   tmpdir=tmpdir)
    partial_sum = np.zeros((T, C), np.float64)
    for r in res.results:
        den = r["out_den"].astype(np.float64).reshape(T, 1)
        partial_sum += r["out"].astype(np.float64) / den
    full = (partial_sum + b_proj[None, :].astype(np.float64)).astype(np.float32)
    return full.reshape(B, N, C), res


def kernel(x, w_qkv, w_proj, b_proj):
    out, _ = run(x, w_qkv, w_proj, b_proj)
    return out
